# revision 2
# baseline (speedup 1.0000x reference)
"""GCN message-passing block on 8 Trainium2 NeuronCores.

Computes: delta = segment_sum((x @ W.T)[source] * edge_weights, target)

Strategy (edge-sharded, fully static SPMD program):
  By linearity, delta = segment_sum(x[source]*w, target) @ W.T -- the node
  projection commutes with the weighted aggregation, so W is applied AFTER
  aggregation (to ~100k rows) instead of per-edge (640k rows).

  Host side: each distinct target node gets a "compacted column". Columns
  are packed CPB=512 per PSUM bank; banks are distributed round-robin over
  the 8 cores. x is split into NCHUNK row-chunks so sources fit int16 for
  the hardware dma_gather. Within a bank, each (chunk c, stripe s) pair
  owns one gather tile of 128 slots; stripe s covers compact columns
  [64s, 64s+64). Edges overflowing their tile are deferred to later banks
  under fresh duplicate columns; the host adds duplicate rows at the end.

  Device side, per bank:
    1. NCHUNK dma_gathers fetch the source rows of x (512B each)
    2. DVE builds per-tile selectors S[e, col] = w_e * (tloc_e == col)
       via an iota-compare (batched over tiles)
    3. per tile: PE matmul Z[:, win] += X_tile.T @ S_tile accumulates the
       weighted segment sums for the bank's columns (dims on partitions)
    4. PE matmul out = Z_slice.T @ W.T flips orientation for free and
       applies the projection; result rows stream to DRAM contiguously.
"""

import numpy as np

import concourse.bacc as bacc
import concourse.bass as bass
import concourse.mybir as mybir
import concourse.tile as tile
from concourse.bass_utils import run_bass_kernel_spmd

N_CORES = 8
NUM_NODES = 100000
D = 128

NCHUNK = 4
CHUNK = NUM_NODES // NCHUNK   # 25000 rows per gather chunk (int16-addressable)
SWIDTH = 64      # columns per stripe == selector window width
NSTR = 8         # stripes per chunk (SWIDTH * NSTR == CPB)
CPB = 512        # compacted columns per PSUM bank (one f32 bank)
SLOT = 128       # gather slots (edges) per tile
NB = 25          # banks per core
TPB = NCHUNK * NSTR            # tiles per bank (32)
SELBATCH = 8     # tiles per selector-build DVE op

NT = NB * TPB          # tiles per core
NCOL = NB * CPB        # output rows (compact columns) per core
NIDX = TPB * SLOT      # gather slots per bank (4096)
F32 = mybir.dt.float32
I16 = mybir.dt.int16


def _mk_ap(base, ap_list):
    return bass.AP(base.tensor, base.offset, ap_list)


def build_program(num_nodes=NUM_NODES, nb=NB, n_cores=N_CORES, stage_bufs=3):
    """Build + compile the single SPMD Bass program (data-independent)."""
    nt = nb * TPB
    ncol = nb * CPB
    chunk = num_nodes // NCHUNK
    nc = bacc.Bacc("TRN2", target_bir_lowering=False, debug=False,
                   num_devices=n_cores)
    x_t = nc.dram_tensor("x", [num_nodes, D], F32, kind="ExternalInput")
    wt_t = nc.dram_tensor("wt", [D, D], F32, kind="ExternalInput")
    # int16 gather indices: per (bank, chunk) a [128, SLOT*NSTR/16] block
    idx_t = nc.dram_tensor("idx16", [SLOT, nb * NCHUNK * (NSTR * SLOT // 16)],
                           I16, kind="ExternalInput")
    tloc_t = nc.dram_tensor("tloc", [SLOT, nt], F32, kind="ExternalInput")
    ew_t = nc.dram_tensor("ew", [SLOT, nt], F32, kind="ExternalInput")
    iota_t = nc.dram_tensor("iota", [SLOT, SWIDTH], F32, kind="ExternalInput")
    out_t = nc.dram_tensor("outc", [ncol, D], F32, kind="ExternalOutput")

    x_ap = x_t.ap()
    out_ap = out_t.ap()
    idxcols = NSTR * SLOT // 16   # 64 idx columns per (bank, chunk)

    with tile.TileContext(nc) as tc:
        with (
            tc.tile_pool(name="const", bufs=1) as constp,
            tc.tile_pool(name="stage", bufs=stage_bufs) as stagep,
            tc.tile_pool(name="sel", bufs=3) as selp,
            tc.tile_pool(name="zsb", bufs=2) as zsbp,
            tc.tile_pool(name="outsb", bufs=2) as outsbp,
            tc.tile_pool(name="psA", bufs=2, space="PSUM") as psA,
            tc.tile_pool(name="psB", bufs=2, space="PSUM") as psB,
        ):
            idx_sb = constp.tile([SLOT, nb * NCHUNK * idxcols], I16)
            tloc_sb = constp.tile([SLOT, nt], F32)
            ew_sb = constp.tile([SLOT, nt], F32)
            iota_sb = constp.tile([SLOT, SWIDTH], F32)
            wt_sb = constp.tile([D, D], F32)
            nc.sync.dma_start(idx_sb[:], idx_t.ap()[:])
            nc.sync.dma_start(tloc_sb[:], tloc_t.ap()[:])
            nc.sync.dma_start(ew_sb[:], ew_t.ap()[:])
            nc.sync.dma_start(iota_sb[:], iota_t.ap()[:])
            nc.sync.dma_start(wt_sb[:], wt_t.ap()[:])

            for b in range(nb):
                t0 = b * TPB
                # 1) gather: one dma_gather per chunk
                xg = stagep.tile([SLOT, TPB * D], F32, tag="xg")
                for c in range(NCHUNK):
                    oslice = xg[:, (c * NSTR) * D:((c + 1) * NSTR) * D]
                    o3 = oslice.rearrange("p (g e) -> p g e", e=D)
                    nc.gpsimd.dma_gather(
                        out_ap=o3,
                        in_ap=x_ap[c * chunk:(c + 1) * chunk, :],
                        idxs_ap=idx_sb[:, (b * NCHUNK + c) * idxcols:
                                       (b * NCHUNK + c + 1) * idxcols],
                        num_idxs=NSTR * SLOT,
                        num_idxs_reg=NSTR * SLOT,
                        elem_size=D,
                    )

                # 2) selector build: S[e, j, col] = ew * (tloc == col)
                sels = []
                for g0 in range(0, TPB, SELBATCH):
                    gn = min(SELBATCH, TPB - g0)
                    S = selp.tile([SLOT, gn * SWIDTH], F32, tag="sel")
                    s3 = S[:].rearrange("p (g w) -> p g w", w=SWIDTH)
                    tl = tloc_sb[:, t0 + g0:t0 + g0 + gn]
                    tl_b = _mk_ap(tl, tl.ap[:2] + [[0, SWIDTH]])
                    io = iota_sb[:]
                    io_b = _mk_ap(io, io.ap[:1] + [[0, gn]] + io.ap[1:])
                    ew = ew_sb[:, t0 + g0:t0 + g0 + gn]
                    ew_b = _mk_ap(ew, ew.ap[:2] + [[0, SWIDTH]])
                    nc.vector.tensor_tensor(
                        out=s3, in0=tl_b, in1=io_b,
                        op=mybir.AluOpType.is_equal)
                    nc.vector.tensor_tensor(
                        out=s3, in0=s3, in1=ew_b,
                        op=mybir.AluOpType.mult)
                    sels.append((g0, S))

                # 3) accumulate weighted segment sums into the PSUM bank
                zp = psA.tile([SLOT, CPB], F32, tag="zp")
                nc.vector.memset(zp[:], 0.0)
                for j in range(TPB):
                    w0 = SWIDTH * (j % NSTR)
                    g0, S = sels[j // SELBATCH]
                    jj = j - g0
                    nc.tensor.matmul(
                        out=zp[:, w0:w0 + SWIDTH],
                        lhsT=xg[:, j * D:(j + 1) * D],
                        rhs=S[:, jj * SWIDTH:(jj + 1) * SWIDTH],
                        start=False, stop=(j == TPB - 1),
                        skip_group_check=True,
                    )

                # 4) apply W.T: out rows (targets) = Z_slice.T @ W.T
                zsb = zsbp.tile([SLOT, CPB], F32, tag="zsb")
                nc.scalar.copy(zsb[:], zp[:])
                ob = psB.tile([SLOT, CPB], F32, tag="ob")
                for q in range(CPB // D):
                    nc.tensor.matmul(
                        out=ob[:, q * D:(q + 1) * D],
                        lhsT=zsb[:, q * D:(q + 1) * D],
                        rhs=wt_sb[:],
                        start=True, stop=True,
                    )
                osb = outsbp.tile([SLOT, CPB], F32, tag="osb")
                nc.scalar.copy(osb[:], ob[:])
                dro = out_ap[b * CPB:(b + 1) * CPB, :].rearrange(
                    "(q p) d -> p q d", p=SLOT)
                sro = osb[:].rearrange("p (q d) -> p q d", d=D)
                nc.sync.dma_start(dro, sro)

    nc.compile()
    return nc


_PROGRAM_CACHE = {}


def _get_program(key="full", **kw):
    if key not in _PROGRAM_CACHE:
        _PROGRAM_CACHE[key] = build_program(**kw)
    return _PROGRAM_CACHE[key]


def preprocess(source, target, edge_weights, num_nodes=NUM_NODES, nb=NB,
               n_cores=N_CORES):
    """Assign edges to (core, bank, chunk, stripe, slot), targets to columns.

    Returns idx16 (replicated int16 gather indices), tloc, ew arrays, the
    column->target map, and leftover edges exceeding capacity (host handles;
    expected empty).
    """
    chunk = num_nodes // NCHUNK
    nt = nb * TPB
    n_banks = nb * n_cores
    idxcols = NSTR * SLOT // 16

    order = np.argsort(target, kind="stable")
    r_src = source[order].astype(np.int64)
    r_tgt = target[order].astype(np.int64)
    r_w = edge_weights[order].astype(np.float32)

    # idx stream per (core, bank, chunk): int16[NSTR*SLOT], default 0
    idxs = np.zeros((n_cores, nb * NCHUNK, NSTR * SLOT), np.int16)
    tloc = np.full((n_cores, SLOT, nt), -1.0, np.float32)
    ewa = np.zeros((n_cores, SLOT, nt), np.float32)
    colmap = np.full((n_cores, nb * CPB), -1, np.int64)

    gb = 0
    leftover = (np.zeros(0, np.int64), np.zeros(0, np.int64),
                np.zeros(0, np.float32))

    while r_tgt.size and gb < n_banks:
        ut, ucnt = np.unique(r_tgt, return_counts=True)
        n_u = ut.size
        ucol = 0
        ecur = 0
        defer = []
        while ucol < n_u and gb < n_banks:
            core = gb % n_cores
            bl = gb // n_cores
            take_u = min(CPB, n_u - ucol)
            bank_ut = ut[ucol:ucol + take_u]
            bank_cnt = ucnt[ucol:ucol + take_u]
            colmap[core, bl * CPB:bl * CPB + take_u] = bank_ut
            e_end = ecur + int(bank_cnt.sum())
            ecol = np.repeat(np.arange(take_u, dtype=np.int64), bank_cnt)
            b_src = r_src[ecur:e_end]
            b_tgt = r_tgt[ecur:e_end]
            b_w = r_w[ecur:e_end]
            b_chunk = b_src // chunk
            b_stripe = ecol // SWIDTH
            # order edges by (chunk, stripe) for grouped slot assignment
            o2 = np.lexsort((b_stripe, b_chunk))
            b_src, b_tgt, b_w = b_src[o2], b_tgt[o2], b_w[o2]
            ecol, b_chunk, b_stripe = ecol[o2], b_chunk[o2], b_stripe[o2]
            key = b_chunk * NSTR + b_stripe
            starts = np.searchsorted(key, np.arange(NCHUNK * NSTR + 1))
            for cs in range(NCHUNK * NSTR):
                lo, hi = int(starts[cs]), int(starts[cs + 1])
                n_e = hi - lo
                if n_e == 0:
                    continue
                c, s = cs // NSTR, cs % NSTR
                k = min(n_e, SLOT)
                sl = slice(lo, lo + k)
                ct = bl * TPB + c * NSTR + s          # tile index in core
                slots = np.arange(k)
                idxs[core, bl * NCHUNK + c, s * SLOT:s * SLOT + k] = (
                    b_src[sl] - c * chunk).astype(np.int16)
                tloc[core, slots, ct] = (ecol[sl] - SWIDTH * s
                                         ).astype(np.float32)
                ewa[core, slots, ct] = b_w[sl]
                if k < n_e:
                    dsl = slice(lo + k, hi)
                    defer.append((b_src[dsl], b_tgt[dsl], b_w[dsl]))
            ucol += take_u
            ecur = e_end
            gb += 1
        if ucol < n_u:
            defer.append((r_src[ecur:], r_tgt[ecur:], r_w[ecur:]))
        if defer:
            r_src = np.concatenate([d[0] for d in defer])
            r_tgt = np.concatenate([d[1] for d in defer])
            r_w = np.concatenate([d[2] for d in defer])
            o3 = np.argsort(r_tgt, kind="stable")
            r_src, r_tgt, r_w = r_src[o3], r_tgt[o3], r_w[o3]
        else:
            r_src = r_tgt = np.zeros(0, np.int64)
            r_w = np.zeros(0, np.float32)
    if r_tgt.size:
        leftover = (r_src, r_tgt, r_w)

    # wrap idx streams into the [128, .../16] int16 layout, 8x replicated
    # stream position i -> [i % 16, i // 16], rows 16k+p replicate row p
    idx16 = np.zeros((n_cores, SLOT, nb * NCHUNK * idxcols), np.int16)
    st = idxs.reshape(n_cores, nb * NCHUNK, NSTR * SLOT // 16, 16)
    for k in range(8):
        idx16[:, 16 * k:16 * (k + 1), :] = (
            st.transpose(0, 3, 1, 2).reshape(n_cores, 16, -1))
    return idx16, tloc, ewa, colmap, leftover


def kernel(x, W, edge_weights, source, target):
    x = np.ascontiguousarray(np.asarray(x, np.float32))
    W = np.asarray(W, np.float32)
    edge_weights = np.asarray(edge_weights, np.float32)
    src = np.asarray(source).astype(np.int64)
    tgt = np.asarray(target).astype(np.int64)
    num_nodes, d = x.shape
    assert d == D and num_nodes == NUM_NODES, (x.shape,)

    idx16, tloc, ewa, colmap, leftover = preprocess(src, tgt, edge_weights)

    nc = _get_program("full")
    wt = np.ascontiguousarray(W.T.astype(np.float32))
    iota = np.broadcast_to(np.arange(SWIDTH, dtype=np.float32),
                           (SLOT, SWIDTH)).copy()
    in_maps = [
        {"x": x, "wt": wt, "idx16": idx16[c], "tloc": tloc[c], "ew": ewa[c],
         "iota": iota}
        for c in range(N_CORES)
    ]
    res = run_bass_kernel_spmd(nc, in_maps, core_ids=list(range(N_CORES)))

    out = np.zeros((num_nodes, D), np.float32)
    all_rows = np.concatenate([res.results[c]["outc"] for c in range(N_CORES)])
    all_cols = colmap.reshape(-1)
    valid = all_cols >= 0
    t_ids = all_cols[valid]
    rows = all_rows[valid]
    uniq, first = np.unique(t_ids, return_index=True)
    out[t_ids[first]] = rows[first]
    dup = np.ones(t_ids.size, bool)
    dup[first] = False
    if dup.any():
        np.add.at(out, t_ids[dup], rows[dup])
    l_src, l_tgt, l_w = leftover
    if l_tgt.size:
        np.add.at(out, l_tgt, (x[l_src] * l_w[:, None]) @ W.T)
    return out


# revision 3
# speedup vs baseline: 9.4604x; 9.4604x over previous
"""GCN message-passing block on 8 Trainium2 NeuronCores.

Computes: delta = segment_sum((x @ W.T)[source] * edge_weights, target)

Strategy (edge-sharded, fully static SPMD program):
  By linearity, delta = segment_sum(x[source]*w, target) @ W.T -- the node
  projection commutes with the weighted aggregation, so W is applied AFTER
  aggregation (to ~100k rows) instead of per-edge (640k rows).

  Host side: each distinct target node gets a "compacted column". Columns
  are packed CPB=512 per PSUM bank; banks are distributed round-robin over
  the 8 cores. x is split into NCHUNK row-chunks so sources fit int16 for
  the hardware dma_gather. Within a bank, each (chunk c, stripe s) pair
  owns one gather tile of 128 slots; stripe s covers compact columns
  [64s, 64s+64). Edges overflowing their tile are deferred to later banks
  under fresh duplicate columns; the host adds duplicate rows at the end.

  Device side, per bank:
    1. NCHUNK dma_gathers fetch the source rows of x (512B each)
    2. DVE builds per-tile selectors S[e, col] = w_e * (tloc_e == col)
       via an iota-compare (batched over tiles)
    3. per tile: PE matmul Z[:, win] += X_tile.T @ S_tile accumulates the
       weighted segment sums for the bank's columns (dims on partitions)
    4. PE matmul out = Z_slice.T @ W.T flips orientation for free and
       applies the projection; result rows stream to DRAM contiguously.
"""

import numpy as np

import concourse.bacc as bacc
import concourse.bass as bass
import concourse.mybir as mybir
import concourse.tile as tile
from concourse.bass_utils import run_bass_kernel_spmd

N_CORES = 8
NUM_NODES = 100000
D = 128

NCHUNK = 4
CHUNK = NUM_NODES // NCHUNK   # 25000 rows per gather chunk (int16-addressable)
SWIDTH = 64      # columns per stripe == selector window width
NSTR = 8         # stripes per chunk (SWIDTH * NSTR == CPB)
CPB = 512        # compacted columns per PSUM bank (one f32 bank)
SLOT = 128       # gather slots (edges) per tile
NB = 25          # banks per core
TPB = NCHUNK * NSTR            # tiles per bank (32)
SELBATCH = 8     # tiles per selector-build DVE op

NT = NB * TPB          # tiles per core
NCOL = NB * CPB        # output rows (compact columns) per core
NIDX = TPB * SLOT      # gather slots per bank (4096)
F32 = mybir.dt.float32
I16 = mybir.dt.int16


def _mk_ap(base, ap_list):
    return bass.AP(base.tensor, base.offset, ap_list)


def build_program(num_nodes=NUM_NODES, nb=NB, n_cores=N_CORES, stage_bufs=3,
                  repeat=1):
    """Build + compile the single SPMD Bass program (data-independent).

    repeat>1 re-runs the whole pipeline (for slope-based benchmarking).
    """
    nt = nb * TPB
    ncol = nb * CPB
    chunk = num_nodes // NCHUNK
    nc = bacc.Bacc("TRN2", target_bir_lowering=False, debug=False,
                   num_devices=n_cores)
    x_t = nc.dram_tensor("x", [num_nodes, D], F32, kind="ExternalInput")
    wt_t = nc.dram_tensor("wt", [D, D], F32, kind="ExternalInput")
    # int16 gather indices: per (bank, chunk) a [128, SLOT*NSTR/16] block
    idx_t = nc.dram_tensor("idx16", [SLOT, nb * NCHUNK * (NSTR * SLOT // 16)],
                           I16, kind="ExternalInput")
    tloc_t = nc.dram_tensor("tloc", [SLOT, nt], F32, kind="ExternalInput")
    ew_t = nc.dram_tensor("ew", [SLOT, nt], F32, kind="ExternalInput")
    iota_t = nc.dram_tensor("iota", [SLOT, SWIDTH], F32, kind="ExternalInput")
    out_t = nc.dram_tensor("outc", [ncol, D], F32, kind="ExternalOutput")

    x_ap = x_t.ap()
    out_ap = out_t.ap()
    idxcols = NSTR * SLOT // 16   # 64 idx columns per (bank, chunk)

    with tile.TileContext(nc) as tc:
        with (
            tc.tile_pool(name="const", bufs=1) as constp,
            tc.tile_pool(name="stage", bufs=stage_bufs) as stagep,
            tc.tile_pool(name="sel", bufs=3) as selp,
            tc.tile_pool(name="zsb", bufs=2) as zsbp,
            tc.tile_pool(name="outsb", bufs=2) as outsbp,
            tc.tile_pool(name="psA", bufs=2, space="PSUM") as psA,
            tc.tile_pool(name="psB", bufs=2, space="PSUM") as psB,
        ):
            idx_sb = constp.tile([SLOT, nb * NCHUNK * idxcols], I16)
            tloc_sb = constp.tile([SLOT, nt], F32)
            ew_sb = constp.tile([SLOT, nt], F32)
            iota_sb = constp.tile([SLOT, SWIDTH], F32)
            wt_sb = constp.tile([D, D], F32)
            nc.sync.dma_start(idx_sb[:], idx_t.ap()[:])
            nc.sync.dma_start(tloc_sb[:], tloc_t.ap()[:])
            nc.sync.dma_start(ew_sb[:], ew_t.ap()[:])
            nc.sync.dma_start(iota_sb[:], iota_t.ap()[:])
            nc.sync.dma_start(wt_sb[:], wt_t.ap()[:])

            for _rep in range(repeat):
              for b in range(nb):
                t0 = b * TPB
                # 1) gather: one dma_gather per chunk
                xg = stagep.tile([SLOT, TPB * D], F32, tag="xg")
                for c in range(NCHUNK):
                    oslice = xg[:, (c * NSTR) * D:((c + 1) * NSTR) * D]
                    o3 = oslice.rearrange("p (g e) -> p g e", e=D)
                    nc.gpsimd.dma_gather(
                        out_ap=o3,
                        in_ap=x_ap[c * chunk:(c + 1) * chunk, :],
                        idxs_ap=idx_sb[:, (b * NCHUNK + c) * idxcols:
                                       (b * NCHUNK + c + 1) * idxcols],
                        num_idxs=NSTR * SLOT,
                        num_idxs_reg=NSTR * SLOT,
                        elem_size=D,
                    )

                # 2) selector build: S[e, j, col] = ew * (tloc == col)
                sels = []
                for g0 in range(0, TPB, SELBATCH):
                    gn = min(SELBATCH, TPB - g0)
                    S = selp.tile([SLOT, gn * SWIDTH], F32, tag="sel")
                    s3 = S[:].rearrange("p (g w) -> p g w", w=SWIDTH)
                    tl = tloc_sb[:, t0 + g0:t0 + g0 + gn]
                    tl_b = _mk_ap(tl, tl.ap[:2] + [[0, SWIDTH]])
                    io = iota_sb[:]
                    io_b = _mk_ap(io, io.ap[:1] + [[0, gn]] + io.ap[1:])
                    ew = ew_sb[:, t0 + g0:t0 + g0 + gn]
                    ew_b = _mk_ap(ew, ew.ap[:2] + [[0, SWIDTH]])
                    nc.vector.tensor_tensor(
                        out=s3, in0=tl_b, in1=io_b,
                        op=mybir.AluOpType.is_equal)
                    nc.vector.tensor_tensor(
                        out=s3, in0=s3, in1=ew_b,
                        op=mybir.AluOpType.mult)
                    sels.append((g0, S))

                # 3) accumulate weighted segment sums into the PSUM bank
                zp = psA.tile([SLOT, CPB], F32, tag="zp")
                nc.vector.memset(zp[:], 0.0)
                for j in range(TPB):
                    w0 = SWIDTH * (j % NSTR)
                    g0, S = sels[j // SELBATCH]
                    jj = j - g0
                    nc.tensor.matmul(
                        out=zp[:, w0:w0 + SWIDTH],
                        lhsT=xg[:, j * D:(j + 1) * D],
                        rhs=S[:, jj * SWIDTH:(jj + 1) * SWIDTH],
                        start=False, stop=(j == TPB - 1),
                        skip_group_check=True,
                    )

                # 4) apply W.T: out rows (targets) = Z_slice.T @ W.T
                zsb = zsbp.tile([SLOT, CPB], F32, tag="zsb")
                nc.scalar.copy(zsb[:], zp[:])
                ob = psB.tile([SLOT, CPB], F32, tag="ob")
                for q in range(CPB // D):
                    nc.tensor.matmul(
                        out=ob[:, q * D:(q + 1) * D],
                        lhsT=zsb[:, q * D:(q + 1) * D],
                        rhs=wt_sb[:],
                        start=True, stop=True,
                    )
                osb = outsbp.tile([SLOT, CPB], F32, tag="osb")
                nc.scalar.copy(osb[:], ob[:])
                dro = out_ap[b * CPB:(b + 1) * CPB, :].rearrange(
                    "(q p) d -> p q d", p=SLOT)
                sro = osb[:].rearrange("p (q d) -> p q d", d=D)
                nc.sync.dma_start(dro, sro)

    nc.compile()
    return nc


_PROGRAM_CACHE = {}


def _get_program(key="full", **kw):
    if key not in _PROGRAM_CACHE:
        _PROGRAM_CACHE[key] = build_program(**kw)
    return _PROGRAM_CACHE[key]


def preprocess(source, target, edge_weights, num_nodes=NUM_NODES, nb=NB,
               n_cores=N_CORES):
    """Assign edges to (core, bank, chunk, stripe, slot), targets to columns.

    Returns idx16 (replicated int16 gather indices), tloc, ew arrays, the
    column->target map, and leftover edges exceeding capacity (host handles;
    expected empty).
    """
    chunk = num_nodes // NCHUNK
    nt = nb * TPB
    n_banks = nb * n_cores
    idxcols = NSTR * SLOT // 16

    order = np.argsort(target, kind="stable")
    r_src = source[order].astype(np.int64)
    r_tgt = target[order].astype(np.int64)
    r_w = edge_weights[order].astype(np.float32)

    # idx stream per (core, bank, chunk): int16[NSTR*SLOT], default 0
    idxs = np.zeros((n_cores, nb * NCHUNK, NSTR * SLOT), np.int16)
    tloc = np.full((n_cores, SLOT, nt), -1.0, np.float32)
    ewa = np.zeros((n_cores, SLOT, nt), np.float32)
    colmap = np.full((n_cores, nb * CPB), -1, np.int64)

    gb = 0
    leftover = (np.zeros(0, np.int64), np.zeros(0, np.int64),
                np.zeros(0, np.float32))

    while r_tgt.size and gb < n_banks:
        ut, ucnt = np.unique(r_tgt, return_counts=True)
        n_u = ut.size
        ucol = 0
        ecur = 0
        defer = []
        while ucol < n_u and gb < n_banks:
            core = gb % n_cores
            bl = gb // n_cores
            take_u = min(CPB, n_u - ucol)
            bank_ut = ut[ucol:ucol + take_u]
            bank_cnt = ucnt[ucol:ucol + take_u]
            colmap[core, bl * CPB:bl * CPB + take_u] = bank_ut
            e_end = ecur + int(bank_cnt.sum())
            ecol = np.repeat(np.arange(take_u, dtype=np.int64), bank_cnt)
            b_src = r_src[ecur:e_end]
            b_tgt = r_tgt[ecur:e_end]
            b_w = r_w[ecur:e_end]
            b_chunk = b_src // chunk
            b_stripe = ecol // SWIDTH
            # order edges by (chunk, stripe) for grouped slot assignment
            o2 = np.lexsort((b_stripe, b_chunk))
            b_src, b_tgt, b_w = b_src[o2], b_tgt[o2], b_w[o2]
            ecol, b_chunk, b_stripe = ecol[o2], b_chunk[o2], b_stripe[o2]
            key = b_chunk * NSTR + b_stripe
            starts = np.searchsorted(key, np.arange(NCHUNK * NSTR + 1))
            for cs in range(NCHUNK * NSTR):
                lo, hi = int(starts[cs]), int(starts[cs + 1])
                n_e = hi - lo
                if n_e == 0:
                    continue
                c, s = cs // NSTR, cs % NSTR
                k = min(n_e, SLOT)
                sl = slice(lo, lo + k)
                ct = bl * TPB + c * NSTR + s          # tile index in core
                slots = np.arange(k)
                idxs[core, bl * NCHUNK + c, s * SLOT:s * SLOT + k] = (
                    b_src[sl] - c * chunk).astype(np.int16)
                tloc[core, slots, ct] = (ecol[sl] - SWIDTH * s
                                         ).astype(np.float32)
                ewa[core, slots, ct] = b_w[sl]
                if k < n_e:
                    dsl = slice(lo + k, hi)
                    defer.append((b_src[dsl], b_tgt[dsl], b_w[dsl]))
            ucol += take_u
            ecur = e_end
            gb += 1
        if ucol < n_u:
            defer.append((r_src[ecur:], r_tgt[ecur:], r_w[ecur:]))
        if defer:
            r_src = np.concatenate([d[0] for d in defer])
            r_tgt = np.concatenate([d[1] for d in defer])
            r_w = np.concatenate([d[2] for d in defer])
            o3 = np.argsort(r_tgt, kind="stable")
            r_src, r_tgt, r_w = r_src[o3], r_tgt[o3], r_w[o3]
        else:
            r_src = r_tgt = np.zeros(0, np.int64)
            r_w = np.zeros(0, np.float32)
    if r_tgt.size:
        leftover = (r_src, r_tgt, r_w)

    # wrap idx streams into the [128, .../16] int16 layout, 8x replicated
    # stream position i -> [i % 16, i // 16], rows 16k+p replicate row p
    idx16 = np.zeros((n_cores, SLOT, nb * NCHUNK * idxcols), np.int16)
    st = idxs.reshape(n_cores, nb * NCHUNK, NSTR * SLOT // 16, 16)
    for k in range(8):
        idx16[:, 16 * k:16 * (k + 1), :] = (
            st.transpose(0, 3, 1, 2).reshape(n_cores, 16, -1))
    return idx16, tloc, ewa, colmap, leftover


def kernel(x, W, edge_weights, source, target):
    x = np.ascontiguousarray(np.asarray(x, np.float32))
    W = np.asarray(W, np.float32)
    edge_weights = np.asarray(edge_weights, np.float32)
    src = np.asarray(source).astype(np.int64)
    tgt = np.asarray(target).astype(np.int64)
    num_nodes, d = x.shape
    assert d == D and num_nodes == NUM_NODES, (x.shape,)

    idx16, tloc, ewa, colmap, leftover = preprocess(src, tgt, edge_weights)

    nc = _get_program("full")
    wt = np.ascontiguousarray(W.T.astype(np.float32))
    iota = np.broadcast_to(np.arange(SWIDTH, dtype=np.float32),
                           (SLOT, SWIDTH)).copy()
    in_maps = [
        {"x": x, "wt": wt, "idx16": idx16[c], "tloc": tloc[c], "ew": ewa[c],
         "iota": iota}
        for c in range(N_CORES)
    ]
    res = run_bass_kernel_spmd(nc, in_maps, core_ids=list(range(N_CORES)))

    out = np.zeros((num_nodes, D), np.float32)
    all_rows = np.concatenate([res.results[c]["outc"] for c in range(N_CORES)])
    all_cols = colmap.reshape(-1)
    valid = all_cols >= 0
    t_ids = all_cols[valid]
    rows = all_rows[valid]
    uniq, first = np.unique(t_ids, return_index=True)
    out[t_ids[first]] = rows[first]
    dup = np.ones(t_ids.size, bool)
    dup[first] = False
    if dup.any():
        np.add.at(out, t_ids[dup], rows[dup])
    l_src, l_tgt, l_w = leftover
    if l_tgt.size:
        np.add.at(out, l_tgt, (x[l_src] * l_w[:, None]) @ W.T)
    return out
